# revision 34
# baseline (speedup 1.0000x reference)
"""GPT-J attention (B=2, S=2048, D=4096, 16 heads x 256, partial RoPE 64) on 8 trn2 cores.

Sharding: tensor-parallel over heads for QKV+attention (each core owns 2 heads:
Wq/Wk/Wv column slices), then an AllToAll converts head-sharding into
sequence-sharding so the out-projection runs with the FULL Wo on a 512-token
shard per core — no ReduceScatter of the 64 MiB partial outputs (the A2A moves
only ~4 MiB of bf16 attention outputs). Each core emits a disjoint
[512, 4096] fp32 output shard; host concatenates.

Device kernel (per core), all matmuls bf16 (fp32 PSUM accumulate):
  - hidden_states pre-transposed on host to hsT [B, D, S] bf16 (contraction on
    partitions).
  - QKV projection per (b, 512-token chunk): three passes (Q, K, V), each
    accumulating all 32 d-chunks directly in PSUM (start/stop over the full
    contraction); weights streamed in 1024-row quarters, hst chunk resident.
    QT/KT produced feature-major [hd, s]; V token-major [s, hd] (stationary /
    moving swapped).
  - RoPE on rot rows via pair-swap PE matmul + DVE mul/add (host-built
    cos/sin with sign folded in), applied per s-chunk right after projection.
  - Attention per (b, head): scores computed TRANSPOSED (ssT [k,q] tiles:
    stationary=KT chunk, moving=QT 512-wide q block) so exp output PT [k, q]
    feeds PV directly as the moving operand (no P transposes, no transpose
    evacuation copies). Causal masks added on diagonal k-chunks (host-built
    transposed patterns). Row sums via a ones[128,128] stationary matmul
    accumulated in PSUM (gives the sum broadcast across partitions for free);
    softmax normalization applied during the PV PSUM->SBUF evacuation
    (tensor_tensor multiply by reciprocal), then DMA straight to the A2A
    input buffer.
  - AllToAll [8 blocks of 512 feat x 512 tok] -> each core holds all 4096
    attention features for its 512 tokens, feature-major.
  - Out-projection: y[tok128, of512] tiles, stationary = z feature chunks,
    moving = full-Wo column blocks streamed (32 MiB bf16, double buffered).
"""

import os
import sys

import numpy as np

sys.path.insert(0, "/opt/trn_rl_repo")

# ---------------------------------------------------------------- constants
B = 2
S = 2048
D = 4096
NH = 16
HD = 256
ROT = 64
MAX_POS = 2048
N_CORES = 8
HPC = NH // N_CORES          # heads per core = 2
HDL = HPC * HD               # local head width = 512

SC = 512                     # s-chunk (projection, q-macro width)
NEG = -1.0e30


def _cfg_full():
    return dict(B=B, S=S, D=D, HPC=HPC, HD=HD, ROT=ROT)


# ---------------------------------------------------------------- bass build

def build_nc(cfg, use_collective=True, n_cores=N_CORES, mm_dtype="bfloat16"):
    import concourse.tile as tile
    from concourse import bacc, bass_isa, mybir

    fp32 = mybir.dt.float32
    bdt = getattr(mybir.dt, mm_dtype)

    Bc, Sc, Dc, HPCc, HDc, ROTc = (
        cfg["B"], cfg["S"], cfg["D"], cfg["HPC"], cfg["HD"], cfg["ROT"])
    HDLc = HPCc * HDc                    # local head width (512)
    NHC = HDLc // 128                    # local hd chunks (4)
    NSC = Sc // SC                       # s-chunks (4)
    NKC = Sc // 128                      # k-chunks per batch (16)
    NQ = 4                               # d-quarters (1024 rows each)
    DCQ = Dc // NQ // 128                # d-chunks per quarter (8)
    SHARD = (Bc * Sc) // n_cores         # tokens per core after A2A (512)
    NTT = SHARD // 128                   # token tiles per core (4)
    NFC = Dc // 128                      # feature chunks (32)
    NOB = Dc // SC                       # out-proj 512-wide blocks (8)

    nc = bacc.Bacc(num_devices=n_cores)

    # inputs (per-core), weight/activation mats pre-tiled on host into SBUF
    # tile layout [.., 128, chunks, free] so each DMA is 128 big descriptors
    # instead of thousands of 1KB ones
    hsT_e = nc.declare_dram_parameter(
        "hsT", [Bc, NSC, NQ, 128, DCQ, SC], bdt, isOutput=False)
    wqT_e = nc.declare_dram_parameter(
        "wqT", [NQ, 128, DCQ, HDLc], bdt, isOutput=False)
    wkT_e = nc.declare_dram_parameter(
        "wkT", [NQ, 128, DCQ, HDLc], bdt, isOutput=False)
    wvT_e = nc.declare_dram_parameter(
        "wvT", [NQ, 128, DCQ, HDLc], bdt, isOutput=False)
    woT_e = nc.declare_dram_parameter(
        "woT", [NOB, 128, NFC, SC], bdt, isOutput=False)
    cos_e = nc.declare_dram_parameter("cosb", [Bc, ROTc, Sc], bdt, isOutput=False)
    sin_e = nc.declare_dram_parameter("sinb", [Bc, ROTc, Sc], bdt, isOutput=False)
    msk_e = nc.declare_dram_parameter("masksT", [128, 128], fp32, isOutput=False)
    psw_e = nc.declare_dram_parameter("pswap", [128, ROTc], bdt, isOutput=False)
    one_e = nc.declare_dram_parameter("ones", [128, 128], bdt, isOutput=False)

    y_e = nc.declare_dram_parameter("y", [SHARD, Dc], fp32, isOutput=True)

    # A2A staging per batch: block j = [512 local feats, 256 toks of batch-b
    # token-chunk j]. Splitting per batch lets A2A(b0) overlap b1 compute.
    TPB = SHARD // Bc                    # tokens per core per batch (256)
    yatt = [nc.dram_tensor(f"yatt{b}", [n_cores * HDLc, TPB], bdt)
            for b in range(Bc)]
    zatt = [nc.dram_tensor(f"zatt{b}", [n_cores * HDLc, TPB], bdt)
            for b in range(Bc)]

    def mm(ps, lhsT, rhs, start, stop):
        nc.tensor.matmul(ps, lhsT, rhs, start=start, stop=stop)

    with tile.TileContext(nc) as tc:
        with (
            tc.tile_pool(name="const", bufs=1) as constp,
            tc.tile_pool(name="zsb", bufs=1) as zp,
        ):
            zt = {}

            def load_z(b):
                # z(b) = token-sharded attention output, feature-major; DMA'd
                # as soon as A2A(b) lands (the pool lives outside the compute
                # pools so these loads never wait on attention SBUF reuse)
                zt[b] = []
                for q in range(NQ):
                    t = zp.tile([128, DCQ, TPB], bdt, tag=f"z{b}{q}",
                                name=f"z{b}{q}")
                    # gpsimd queue: these gather-pattern DMAs cost ~2-6us of
                    # descriptor writes each; on sync they head-block the
                    # weight-stream triggers at the phase transitions
                    nc.gpsimd.dma_start(
                        t[:],
                        zatt[b][q * 1024:(q + 1) * 1024, :]
                        .rearrange("(j p) f -> p j f", p=128))
                    zt[b].append(t)

            with (
                tc.tile_pool(name="qkv", bufs=1) as qkvp,
                tc.tile_pool(name="hst", bufs=6) as hstp,
                tc.tile_pool(name="wst", bufs=4) as wstp,
                tc.tile_pool(name="rope", bufs=2) as ropep,
                tc.tile_pool(name="ptsb", bufs=8) as ptp,
                tc.tile_pool(name="rcp", bufs=2) as rcpp,
                tc.tile_pool(name="atnsb", bufs=2) as atnp,
            ):
                def load_hq(b, sc):
                    hq = []
                    for q in range(NQ):
                        ht = hstp.tile([128, DCQ, SC], bdt, tag="h", name=f"h{q}")
                        nc.sync.dma_start(ht[:], hsT_e[b, sc, q])
                        hq.append(ht)
                    return hq

                # rolling weight-quarter prefetcher: the whole phase-A program
                # consumes (wq, wk, wv) x 4 quarters per (b, sc) in a fixed
                # order; keep 2 quarters in flight ahead of the consumer so a
                # pass never waits on its weight DMA (and b1's first quarter
                # is issued during b0's attention, ahead of the attention
                # output DMAs in the queues).
                from collections import deque
                w_seq = deque()
                for b in range(Bc):
                    for sc in range(NSC):
                        for w_e in (wqT_e, wkT_e, wvT_e):
                            for q in range(NQ):
                                w_seq.append((w_e, q))
                w_pending = deque()

                def w_issue():
                    if not w_seq:
                        return
                    w_e, q = w_seq.popleft()
                    wt = wstp.tile([128, DCQ, HDLc], bdt, tag="w")
                    nc.sync.dma_start(wt[:], w_e[q])
                    w_pending.append(wt)

                def w_pop():
                    w_issue()
                    return w_pending.popleft()

                # interleave the first hst quarters with the first weight
                # quarters so each projection quarter's dependency pair (h_q +
                # w_q) lands in consumption order in the in-order DMA queues;
                # the very first quarter is split in half so the opening
                # matmuls start after 1.5 MiB instead of 2 MiB
                h0ab = []
                for hh in range(2):
                    t = hstp.tile([128, DCQ // 2, SC], bdt, tag="h",
                                  name=f"h0{hh}")
                    nc.sync.dma_start(
                        t[:], hsT_e[0, 0, 0][:, hh * 4:(hh + 1) * 4, :])
                    h0ab.append(t)
                    if hh == 0:
                        w_issue()
                first_hq = [("split", h0ab)]
                for q in range(1, NQ):
                    ht = hstp.tile([128, DCQ, SC], bdt, tag="h", name=f"h{q}")
                    nc.sync.dma_start(ht[:], hsT_e[0, 0, q])
                    first_hq.append(ht)
                    if q < 3:
                        w_issue()
                next_hq = first_hq
                # const tiles allocated here, but their DMAs are emitted after
                # the first Q-pass so the opening weight stream isn't queued
                # behind them (cos/sin are first needed ~45us in, masks/ones
                # only at attention). pswap is dead since RoPE moved to DMA.
                masksT = constp.tile([128, 128], fp32)
                ones_t = constp.tile([128, 128], bdt)
                cosb = [constp.tile([ROTc, Sc], bdt, name=f"cos{b}")
                        for b in range(Bc)]
                sinb = [constp.tile([ROTc, Sc], bdt, name=f"sin{b}")
                        for b in range(Bc)]

                def load_consts():
                    nc.sync.dma_start(masksT[:], msk_e[:])
                    nc.sync.dma_start(ones_t[:], one_e[:])
                    for bb in range(Bc):
                        nc.sync.dma_start(cosb[bb][:], cos_e[bb])
                        nc.sync.dma_start(sinb[bb][:], sin_e[bb])

                # out-proj weight blocks are loaded into freed hst/wst pool
                # buffers (identical tile shape) — no extra SBUF. Splitting
                # across both pools gives 10 effective buffers, enough to
                # fully double-buffer a 4-tile block per phase-C visit.
                wo_tiles = {}

                def load_wo(ob):
                    tiles = []
                    for q in range(NQ):
                        pool, tg = (hstp, "h") if q < 2 else (wstp, "w")
                        w4 = pool.tile([128, DCQ, SC], bdt, tag=tg,
                                       name=f"wo{q}")
                        nc.sync.dma_start(
                            w4[:], woT_e[ob, :, q * DCQ:(q + 1) * DCQ, :])
                        tiles.append(w4)
                    return tiles

                for b in range(Bc):
                    QT = [qkvp.tile([128, Sc], bdt, tag=f"QT{c}", name=f"QT{c}")
                          for c in range(NHC)]
                    KT = [qkvp.tile([128, Sc], bdt, tag=f"KT{c}", name=f"KT{c}")
                          for c in range(NHC)]
                    V = [qkvp.tile([128, HDLc], bdt, tag=f"V{k}", name=f"V{k}")
                         for k in range(NKC)]

                    # ---------------- phase A: QKV projection + RoPE ----------------
                    with tc.tile_pool(name="pjps", bufs=2, space="PSUM") as pjps:
                        for sc in range(NSC):
                            ssl = slice(sc * SC, (sc + 1) * SC)
                            hq = next_hq
                            if sc + 1 < NSC:
                                next_hq = load_hq(b, sc + 1)
                            elif b + 1 < Bc:
                                next_hq = load_hq(b + 1, 0)

                            def hqs(q, dc, csl=slice(0, SC)):
                                ent = hq[q]
                                if isinstance(ent, tuple):
                                    return ent[1][dc // 4][:, dc % 4, csl]
                                return ent[:, dc, csl]

                            # RoPE on rot rows: the adjacent-partition swap is
                            # done by two SBUF->SBUF DMAs on the idle scalar
                            # queue (the data is already bf16, so this is
                            # exact and frees ~10us of PE matmuls), then DVE
                            # mul/add. Emitted one quarter INTO the following
                            # pass so the serial DVE chain hides under that
                            # pass's matmuls instead of stalling attention.
                            def rope_block(T, ti):
                                for hch in range(0, NHC, HDc // 128):
                                    sw = ropep.tile([ROTc, SC], bdt, tag="sw")
                                    src = (T[hch][0:ROTc, ssl]
                                           .rearrange("(a two) f -> a two f",
                                                      two=2))
                                    dst = sw[:].rearrange(
                                        "(a two) f -> a two f", two=2)
                                    nc.scalar.dma_start(dst[:, 0, :],
                                                        src[:, 1, :])
                                    nc.scalar.dma_start(dst[:, 1, :],
                                                        src[:, 0, :])
                                    t1 = ropep.tile([ROTc, SC], bdt, tag="t1")
                                    t2 = ropep.tile([ROTc, SC], bdt, tag="t2")
                                    nc.vector.tensor_tensor(
                                        t1[:], sw[:], sinb[b][:, ssl],
                                        op=mybir.AluOpType.mult)
                                    nc.vector.tensor_tensor(
                                        t2[:], T[hch][0:ROTc, ssl], cosb[b][:, ssl],
                                        op=mybir.AluOpType.mult)
                                    nc.vector.tensor_add(T[hch][0:ROTc, ssl],
                                                         t1[:], t2[:])

                            # Q and K passes: out [hd=128, s=512] per head-chunk
                            for pi, (w_e, T) in enumerate(((wqT_e, QT),
                                                           (wkT_e, KT))):
                                ps = [pjps.tile([128, SC], fp32, tag=f"pj{i}",
                                                name=f"pj{i}")
                                      for i in range(NHC)]
                                for q in range(NQ):
                                    wt = w_pop()
                                    for dc in range(DCQ):
                                        for hc in range(NHC):
                                            mm(ps[hc][:],
                                               wt[:, dc, hc * 128:(hc + 1) * 128],
                                               hqs(q, dc),
                                               start=(q == 0 and dc == 0),
                                               stop=(q == NQ - 1 and dc == DCQ - 1))
                                    if q == 0 and pi == 1:
                                        rope_block(QT, 0)   # hides in K pass
                                    if q == NQ - 1 and pi == 0 \
                                            and b == 0 and sc == 0:
                                        load_consts()
                                # alternate engines: the next pass's PSUM
                                # buffer rotation waits on these evacuations
                                for hc in range(NHC):
                                    if hc % 2 == 0:
                                        nc.scalar.copy(T[hc][:, ssl],
                                                       ps[hc][:])
                                    else:
                                        nc.vector.tensor_copy(T[hc][:, ssl],
                                                              ps[hc][:])

                            # V pass: out [s=128, hdl=512] per token subtile.
                            # Tags offset by 2: the ropeQ sw tiles took pj0/pj1
                            # (Q-pass buffers); mapping ts 0/1 onto pj2/pj3
                            # (also Q-pass buffers, long free) keeps the first
                            # V matmuls off the just-retired K-pass buffers,
                            # whose scalar-copy evacuations are still running.
                            ps = [pjps.tile([128, HDLc], fp32,
                                            tag=f"pj{(i + 2) % 4}",
                                            name=f"pv{i}")
                                  for i in range(NHC)]
                            for q in range(NQ):
                                wt = w_pop()
                                for dc in range(DCQ):
                                    for ts in range(4):
                                        mm(ps[ts][:],
                                           hqs(q, dc,
                                               slice(ts * 128, (ts + 1) * 128)),
                                           wt[:, dc, :],
                                           start=(q == 0 and dc == 0),
                                           stop=(q == NQ - 1 and dc == DCQ - 1))
                                if q == 0:
                                    rope_block(KT, 1)       # hides in V pass
                            # alternate engines: these four evacuations gate
                            # the pjps pool-close barrier ahead of attention
                            for ts in range(4):
                                if ts % 2 == 0:
                                    nc.vector.tensor_copy(V[sc * 4 + ts][:],
                                                          ps[ts][:])
                                else:
                                    nc.scalar.copy(V[sc * 4 + ts][:], ps[ts][:])

                    # ---------------- phase B: attention ----------------
                    if b == 1:
                        load_z(0)   # A2A(b0) is done by now; overlaps b1 work
                        # prefetch the first two out-proj weight blocks during
                        # b1's attention so phase C starts without DMA stalls;
                        # the tiles reuse freed hst/wst pool buffers
                        wo_tiles[0] = load_wo(0)
                        wo_tiles[1] = load_wo(1)
                    with (
                        tc.tile_pool(name="ssps", bufs=3, space="PSUM") as ssps,
                        tc.tile_pool(name="atps0", bufs=2, space="PSUM") as atps0,
                        tc.tile_pool(name="atps1", bufs=2, space="PSUM") as atps1,
                        tc.tile_pool(name="rsps", bufs=1, space="PSUM") as rsps,
                    ):
                        for h in range(HPCc):
                            c0, c1 = 2 * h, 2 * h + 1
                            for qm in range(NSC):
                                nkc = 4 * (qm + 1)
                                npair = nkc // 2
                                LAG = 3          # PV lag in kc-pairs

                                # diagonal trimming: for the last 4 k-chunks,
                                # q-columns below the diagonal block are fully
                                # masked — skip them. Computed region of kc =
                                # absolute q [off(kc), 512): off = 128 *
                                # (kc - (nkc-4)) for diagonal kcs else 0.
                                def off(kc, _n=nkc):
                                    return max(0, (kc - (_n - 4))) * 128

                                at_ps = [
                                    atps0.tile([128, SC], fp32, tag="at0",
                                               name="at0"),
                                    atps1.tile([128, SC], fp32, tag="at1",
                                               name="at1"),
                                ]
                                rs_ps = rsps.tile([128, SC], fp32, tag="rs")
                                pts = {}
                                # kc processed in pairs, banks interleaved so
                                # no two consecutive matmuls hit the same
                                # PSUM bank; PV lags LAG pairs behind so exp
                                # (ACT) latency stays off the PE critical
                                # path. Row sums via ones-stationary matmul
                                # (result broadcast across partitions free).
                                for step in range(npair + LAG):
                                    if step < npair:
                                        k0, k1 = 2 * step, 2 * step + 1
                                        o0, o1 = off(k0), off(k1)
                                        w0, w1 = SC - o0, SC - o1
                                        l0 = slice(k0 * 128, (k0 + 1) * 128)
                                        l1 = slice(k1 * 128, (k1 + 1) * 128)
                                        q0 = slice(qm * SC + o0, (qm + 1) * SC)
                                        q1 = slice(qm * SC + o1, (qm + 1) * SC)
                                        s0 = ssps.tile([128, SC], fp32, tag="ss",
                                                       name="ss0")
                                        s1 = ssps.tile([128, SC], fp32, tag="ss",
                                                       name="ss1")
                                        mm(s0[:, 0:w0], KT[c0][:, l0],
                                           QT[c0][:, q0], start=True, stop=False)
                                        mm(s1[:, 0:w1], KT[c0][:, l1],
                                           QT[c0][:, q1], start=True, stop=False)
                                        mm(s0[:, 0:w0], KT[c1][:, l0],
                                           QT[c1][:, q0], start=False, stop=True)
                                        mm(s1[:, 0:w1], KT[c1][:, l1],
                                           QT[c1][:, q1], start=False, stop=True)
                                        for kc, ss in ((k0, s0), (k1, s1)):
                                            o, w = off(kc), SC - off(kc)
                                            if kc >= nkc - 4:
                                                # triangle sits in the first
                                                # 128 computed columns
                                                nc.vector.tensor_add(
                                                    ss[:, 0:128], ss[:, 0:128],
                                                    masksT[:])
                                            pt = ptp.tile([128, SC], bdt,
                                                          tag="pt")
                                            nc.scalar.activation(
                                                pt[:, 0:w], ss[:, 0:w],
                                                mybir.ActivationFunctionType.Exp,
                                                bias=0.0, scale=1.0 / 16.0)
                                            pts[kc] = pt
                                    if step >= LAG:
                                        for kc in (2 * (step - LAG),
                                                   2 * (step - LAG) + 1):
                                            pt = pts.pop(kc)
                                            o, w = off(kc), SC - off(kc)
                                            st = (kc == 0)
                                            sp = (kc == nkc - 1)
                                            mm(rs_ps[:, o:], ones_t[:],
                                               pt[:, 0:w], start=st, stop=sp)
                                            mm(at_ps[0][:, o:],
                                               V[kc][:, h * HDc:h * HDc + 128],
                                               pt[:, 0:w], start=st, stop=sp)
                                            mm(at_ps[1][:, o:],
                                               V[kc][:, h * HDc + 128:(h + 1) * HDc],
                                               pt[:, 0:w], start=st, stop=sp)
                                recip = rcpp.tile([128, SC], fp32, tag="rc")
                                # exact reciprocal costs 3.4us on DVE and
                                # head-blocks the mask-adds/normalizes queued
                                # behind it at every qm boundary; the ~18-bit
                                # approx is 5x faster and the denominator is a
                                # benign [1, ~5e2] softmax rowsum
                                nc.vector.reciprocal_approx_fast(
                                    recip[:], rs_ps[:])
                                for hh in range(2):
                                    atn = atnp.tile([128, SC], bdt, tag=f"atn{hh}")
                                    nc.vector.tensor_tensor(
                                        atn[:], at_ps[hh][:], recip[:],
                                        op=mybir.AluOpType.mult)
                                    row0 = h * HDc + hh * 128
                                    # split across the two destination cores
                                    # covering this 512-token q block; gpsimd
                                    # queue (same as the consuming A2A) keeps
                                    # these off the sync queue's weight-stream
                                    # triggers
                                    for half in range(2):
                                        j = 2 * qm + half
                                        nc.gpsimd.dma_start(
                                            yatt[b][j * HDLc + row0:
                                                    j * HDLc + row0 + 128, :],
                                            atn[:, half * TPB:(half + 1) * TPB])

                    # A2A(b): head-sharded -> token-sharded; b0's overlaps
                    # b1's QKV/attention compute entirely.
                    nc.gpsimd.collective_compute(
                        "AllToAll",
                        mybir.AluOpType.bypass,
                        replica_groups=[list(range(n_cores))],
                        ins=[yatt[b][:]],
                        outs=[zatt[b][:]],
                    )
                    if b == 1:
                        # z(b1) pull is dependency-tracked on the A2A write;
                        # issuing it here puts it ahead of phase C's weight
                        # streaming in the DMA queues
                        load_z(1)

                # ---------------- phase C: out projection ----------------
                # Wo column-blocks mostly stream ONCE and serve both batches,
                # but the A2A(b1) collective takes ~40us after b1's attention
                # drains — so the first DEFER obs visit only b0's tokens
                # (their z landed with A2A(b0) long ago) and their b1 halves
                # run at the very end (re-streaming just DEFER x 4 MiB of Wo).
                DEFER = 4
                visits = ([(ob, (0,)) for ob in range(DEFER)]
                          + [(ob, (0, 1)) for ob in range(DEFER, NOB)]
                          + [(ob, (1,)) for ob in range(DEFER)])
                with (
                    tc.tile_pool(name="ysb", bufs=4) as ysbp,
                    tc.tile_pool(name="yps", bufs=4, space="PSUM") as ypsp,
                ):
                    for vi, (ob, bs) in enumerate(visits):
                        ocl = slice(ob * SC, (ob + 1) * SC)
                        woq = (wo_tiles.pop(ob) if ob in wo_tiles
                               else load_wo(ob))
                        if vi + 1 < len(visits):
                            nob = visits[vi + 1][0]
                            if nob not in wo_tiles:
                                wo_tiles[nob] = load_wo(nob)
                        for b in bs:
                            z = zt[b]
                            # 2 token-tile accumulation chains interleaved so
                            # consecutive matmuls hit different PSUM banks
                            yp = [ypsp.tile([128, SC], fp32, tag="yp",
                                            name=f"yp{tt}")
                                  for tt in range(TPB // 128)]
                            for fc in range(NFC):
                                for tt in range(TPB // 128):
                                    mm(yp[tt][:],
                                       z[fc // DCQ][:, fc % DCQ,
                                                    tt * 128:(tt + 1) * 128],
                                       woq[fc // DCQ][:, fc % DCQ, :],
                                       start=(fc == 0), stop=(fc == NFC - 1))
                            for tt in range(TPB // 128):
                                tsl = slice(b * TPB + tt * 128,
                                            b * TPB + (tt + 1) * 128)
                                ysb = ysbp.tile([128, SC], fp32, tag="ysb")
                                if tt % 2 == 0:
                                    nc.scalar.copy(ysb[:], yp[tt][:])
                                else:
                                    nc.vector.tensor_copy(ysb[:], yp[tt][:])
                                nc.sync.dma_start(y_e[tsl, ocl], ysb[:])

    nc.compile()
    return nc


# ---------------------------------------------------------------- host prep

def _sinusoidal_np(num_pos, dim):
    inv_freq = 1.0 / (10000.0 ** (np.arange(0, dim, 2, dtype=np.float32) / dim))
    t = np.arange(num_pos, dtype=np.float32)[:, None] * inv_freq[None, :]
    return np.cos(t).astype(np.float32), np.sin(t).astype(np.float32)  # [P, dim//2]


def _host_arrays(hs, Wq, Wk, Wv, Wo, position_ids, cfg, n_cores):
    """Build the shared + per-core input arrays."""
    import ml_dtypes
    bf16 = ml_dtypes.bfloat16

    Bc, Sc, Dc, HPCc, HDc, ROTc = (
        cfg["B"], cfg["S"], cfg["D"], cfg["HPC"], cfg["HD"], cfg["ROT"])
    HDLc = HPCc * HDc
    NSC, NQ = Sc // SC, 4
    DCQ = Dc // NQ // 128
    NOB, NFC = Dc // SC, Dc // 128
    # hsT pre-tiled to [B, sc, q, 128, dcq, 512] (SBUF tile layout)
    hsT = np.ascontiguousarray(hs.transpose(0, 2, 1)).astype(bf16)  # [B, D, S]
    hsT = hsT.reshape(Bc, NQ, DCQ, 128, NSC, SC).transpose(0, 4, 1, 3, 2, 5)
    hsT = np.ascontiguousarray(hsT)

    def tile_w(wT):      # [D, 512] -> [q, 128, dcq, 512]
        return np.ascontiguousarray(
            wT.reshape(NQ, DCQ, 128, HDLc).transpose(0, 2, 1, 3))

    cos_t, sin_t = _sinusoidal_np(max(MAX_POS, Sc), ROTc)   # [P, ROT//2]
    pos = np.asarray(position_ids).astype(np.int64)         # [B, S]
    cosg = cos_t[pos]                                       # [B, S, 32]
    sing = sin_t[pos]
    cosb = np.repeat(cosg.transpose(0, 2, 1), 2, axis=1)    # [B, 64, S]
    sinb_r = np.repeat(sing.transpose(0, 2, 1), 2, axis=1)
    sgn = np.ones((ROTc, 1), np.float32)
    sgn[0::2] = -1.0
    sinb = np.ascontiguousarray(sinb_r * sgn).astype(bf16)
    cosb = np.ascontiguousarray(cosb).astype(bf16)

    # transposed causal triangle for a diagonal 128x128 block:
    # masksT[r, c] = 0 if r <= c else NEG  (k_local <= q_local)
    kk = np.arange(128)[:, None]
    qq = np.arange(128)[None, :]
    masksT = np.where(kk <= qq, 0.0, NEG).astype(np.float32)

    pswap = np.zeros((128, ROTc), np.float32)
    for f in range(ROTc // 2):
        pswap[2 * f + 1, 2 * f] = 1.0
        pswap[2 * f, 2 * f + 1] = 1.0
    pswap = pswap.astype(bf16)
    ones = np.ones((128, 128), np.float32).astype(bf16)

    woT = np.ascontiguousarray(np.asarray(Wo, np.float32).T).astype(bf16)
    # pre-tiled to [ob, 128, fc, 512]
    woT = np.ascontiguousarray(
        woT.reshape(NFC, 128, NOB, SC).transpose(2, 1, 0, 3))

    shared = dict(hsT=hsT, cosb=cosb, sinb=sinb, masksT=masksT,
                  pswap=pswap, ones=ones, woT=woT)
    per_core = []
    for c in range(n_cores):
        csl = slice(c * HDLc, (c + 1) * HDLc)
        per_core.append(dict(
            wqT=tile_w(np.ascontiguousarray(Wq[csl, :].T).astype(bf16)),
            wkT=tile_w(np.ascontiguousarray(Wk[csl, :].T).astype(bf16)),
            wvT=tile_w(np.ascontiguousarray(Wv[csl, :].T).astype(bf16)),
            **shared,
        ))
    return per_core


def _numpy_reference(hidden_states, Wq, Wk, Wv, Wo, layer_past_k, layer_past_v,
                     attention_mask, position_ids, new_key_loc, new_value_loc,
                     valid_key_indices, valid_value_indices, bucket_size):
    """Slow but general fallback (mirrors reference.py in numpy fp32)."""
    hs = np.asarray(hidden_states, np.float32)
    Bc, Sc, Dc = hs.shape
    q = (hs @ np.asarray(Wq).T).reshape(Bc, Sc, NH, HD)
    k = (hs @ np.asarray(Wk).T).reshape(Bc, Sc, NH, HD)
    v = (hs @ np.asarray(Wv).T).reshape(Bc, Sc, NH, HD)

    cos_t, sin_t = _sinusoidal_np(MAX_POS, ROT)
    pos = np.asarray(position_ids).astype(np.int64)
    c_ = cos_t[pos][:, :, None, :]      # [B,S,1,32]
    s_ = sin_t[pos][:, :, None, :]

    def rot(x):
        xr = x[..., :ROT].reshape(Bc, Sc, NH, ROT // 2, 2)
        x0, x1 = xr[..., 0], xr[..., 1]
        o0 = c_ * x0 - s_ * x1
        o1 = s_ * x0 + c_ * x1
        out = np.stack([o0, o1], axis=-1).reshape(Bc, Sc, NH, ROT)
        return np.concatenate([out, x[..., ROT:]], axis=-1)

    q, k = rot(q), rot(k)
    nk = np.asarray(layer_past_k, np.float32).copy()
    nv = np.asarray(layer_past_v, np.float32).copy()
    nk[np.asarray(new_key_loc)] = k.reshape(Bc * Sc, 1, NH, HD)
    nv[np.asarray(new_value_loc)] = v.reshape(Bc * Sc, 1, NH, HD)
    kg = nk[np.asarray(valid_key_indices)].reshape(
        Bc, bucket_size, NH, HD).transpose(0, 2, 1, 3)
    vg = nv[np.asarray(valid_value_indices)].reshape(
        Bc, bucket_size, NH, HD).transpose(0, 2, 1, 3)
    qh = q.transpose(0, 2, 1, 3)
    scores = np.einsum("bhqd,bhkd->bhqk", qh, kg)
    causal = np.tril(np.ones((MAX_POS, MAX_POS), bool))[
        bucket_size - Sc:bucket_size, :bucket_size]
    scores = np.where(causal, scores, np.float32(np.finfo(np.float32).min))
    scores = scores / np.float32(np.sqrt(HD)) + np.asarray(attention_mask, np.float32)
    scores = scores - scores.max(-1, keepdims=True)
    p = np.exp(scores)
    p = p / p.sum(-1, keepdims=True)
    attn = np.einsum("bhqk,bhkd->bhqd", p, vg)
    attn = attn.transpose(0, 2, 1, 3).reshape(Bc, Sc, Dc)
    return (attn @ np.asarray(Wo).T).astype(np.float32)


def _fast_path_ok(layer_past_k, layer_past_v, attention_mask, new_key_loc,
                  new_value_loc, valid_key_indices, valid_value_indices,
                  bucket_size, hs_shape):
    Bc, Sc, Dc = hs_shape
    if (Bc, Sc, Dc) != (B, S, D) or int(bucket_size) != S:
        return False
    ar = np.arange(Bc * Sc)
    for idx in (new_key_loc, new_value_loc, valid_key_indices, valid_value_indices):
        a = np.asarray(idx)
        if a.shape != (Bc * Sc,) or not np.array_equal(a, ar):
            return False
    if np.any(np.asarray(attention_mask) != 0):
        return False
    return True


_NC_CACHE = {}


def _get_nc(use_collective=True):
    key = "full"
    if key not in _NC_CACHE:
        _NC_CACHE[key] = build_nc(_cfg_full(), n_cores=N_CORES)
    return _NC_CACHE[key]


def kernel(**inputs):
    hs = np.asarray(inputs["hidden_states"], np.float32)
    fast = _fast_path_ok(
        inputs["layer_past_k"], inputs["layer_past_v"], inputs["attention_mask"],
        inputs["new_key_loc"], inputs["new_value_loc"],
        inputs["valid_key_indices"], inputs["valid_value_indices"],
        inputs["bucket_size"], hs.shape)
    if not fast:
        return _numpy_reference(**inputs)

    from concourse.bass_utils import run_bass_kernel_spmd

    nc = _get_nc(True)
    in_maps = _host_arrays(
        hs, np.asarray(inputs["Wq"], np.float32),
        np.asarray(inputs["Wk"], np.float32),
        np.asarray(inputs["Wv"], np.float32),
        np.asarray(inputs["Wo"], np.float32),
        inputs["position_ids"], _cfg_full(), N_CORES)
    res = run_bass_kernel_spmd(nc, in_maps, list(range(N_CORES)))
    outs = [res.results[c]["y"] for c in range(N_CORES)]
    return _unshard(outs)


def _unshard(outs):
    """Core c's [512, D] shard = [b0 tokens 256c:256(c+1); b1 same range]."""
    tpb = (B * S) // N_CORES // B        # 256
    y = np.empty((B, S, D), np.float32)
    for c, o in enumerate(outs):
        for b in range(B):
            y[b, c * tpb:(c + 1) * tpb] = o[b * tpb:(b + 1) * tpb]
    return y



# revision 38
# speedup vs baseline: 1.0020x; 1.0020x over previous
"""GPT-J attention (B=2, S=2048, D=4096, 16 heads x 256, partial RoPE 64) on 8 trn2 cores.

Sharding: tensor-parallel over heads for QKV+attention (each core owns 2 heads:
Wq/Wk/Wv column slices), then an AllToAll converts head-sharding into
sequence-sharding so the out-projection runs with the FULL Wo on a 512-token
shard per core — no ReduceScatter of the 64 MiB partial outputs (the A2A moves
only ~4 MiB of bf16 attention outputs). Each core emits a disjoint
[512, 4096] fp32 output shard; host concatenates.

Device kernel (per core), all matmuls bf16 (fp32 PSUM accumulate):
  - hidden_states pre-transposed on host to hsT [B, D, S] bf16 (contraction on
    partitions).
  - QKV projection per (b, 512-token chunk): three passes (Q, K, V), each
    accumulating all 32 d-chunks directly in PSUM (start/stop over the full
    contraction); weights streamed in 1024-row quarters, hst chunk resident.
    QT/KT produced feature-major [hd, s]; V token-major [s, hd] (stationary /
    moving swapped).
  - RoPE on rot rows via pair-swap PE matmul + DVE mul/add (host-built
    cos/sin with sign folded in), applied per s-chunk right after projection.
  - Attention per (b, head): scores computed TRANSPOSED (ssT [k,q] tiles:
    stationary=KT chunk, moving=QT 512-wide q block) so exp output PT [k, q]
    feeds PV directly as the moving operand (no P transposes, no transpose
    evacuation copies). Causal masks added on diagonal k-chunks (host-built
    transposed patterns). Row sums via a ones[128,128] stationary matmul
    accumulated in PSUM (gives the sum broadcast across partitions for free);
    softmax normalization applied during the PV PSUM->SBUF evacuation
    (tensor_tensor multiply by reciprocal), then DMA straight to the A2A
    input buffer.
  - AllToAll [8 blocks of 512 feat x 512 tok] -> each core holds all 4096
    attention features for its 512 tokens, feature-major.
  - Out-projection: y[tok128, of512] tiles, stationary = z feature chunks,
    moving = full-Wo column blocks streamed (32 MiB bf16, double buffered).
"""

import os
import sys

import numpy as np

sys.path.insert(0, "/opt/trn_rl_repo")

# ---------------------------------------------------------------- constants
B = 2
S = 2048
D = 4096
NH = 16
HD = 256
ROT = 64
MAX_POS = 2048
N_CORES = 8
HPC = NH // N_CORES          # heads per core = 2
HDL = HPC * HD               # local head width = 512

SC = 512                     # s-chunk (projection, q-macro width)
NEG = -1.0e30


def _cfg_full():
    return dict(B=B, S=S, D=D, HPC=HPC, HD=HD, ROT=ROT)


# ---------------------------------------------------------------- bass build

def build_nc(cfg, use_collective=True, n_cores=N_CORES, mm_dtype="bfloat16"):
    import concourse.tile as tile
    from concourse import bacc, bass_isa, mybir

    fp32 = mybir.dt.float32
    bdt = getattr(mybir.dt, mm_dtype)

    Bc, Sc, Dc, HPCc, HDc, ROTc = (
        cfg["B"], cfg["S"], cfg["D"], cfg["HPC"], cfg["HD"], cfg["ROT"])
    HDLc = HPCc * HDc                    # local head width (512)
    NHC = HDLc // 128                    # local hd chunks (4)
    NSC = Sc // SC                       # s-chunks (4)
    NKC = Sc // 128                      # k-chunks per batch (16)
    NQ = 4                               # d-quarters (1024 rows each)
    DCQ = Dc // NQ // 128                # d-chunks per quarter (8)
    SHARD = (Bc * Sc) // n_cores         # tokens per core after A2A (512)
    NTT = SHARD // 128                   # token tiles per core (4)
    NFC = Dc // 128                      # feature chunks (32)
    NOB = Dc // SC                       # out-proj 512-wide blocks (8)

    nc = bacc.Bacc(num_devices=n_cores)

    # inputs (per-core), weight/activation mats pre-tiled on host into SBUF
    # tile layout [.., 128, chunks, free] so each DMA is 128 big descriptors
    # instead of thousands of 1KB ones
    hsT_e = nc.declare_dram_parameter(
        "hsT", [Bc, NSC, NQ, 128, DCQ, SC], bdt, isOutput=False)
    wqT_e = nc.declare_dram_parameter(
        "wqT", [NQ, 128, DCQ, HDLc], bdt, isOutput=False)
    wkT_e = nc.declare_dram_parameter(
        "wkT", [NQ, 128, DCQ, HDLc], bdt, isOutput=False)
    wvT_e = nc.declare_dram_parameter(
        "wvT", [NQ, 128, DCQ, HDLc], bdt, isOutput=False)
    woT_e = nc.declare_dram_parameter(
        "woT", [NOB, 128, NFC, SC], bdt, isOutput=False)
    cos_e = nc.declare_dram_parameter("cosb", [Bc, ROTc, Sc], bdt, isOutput=False)
    sin_e = nc.declare_dram_parameter("sinb", [Bc, ROTc, Sc], bdt, isOutput=False)
    msk_e = nc.declare_dram_parameter("masksT", [128, 128], fp32, isOutput=False)
    psw_e = nc.declare_dram_parameter("pswap", [128, ROTc], bdt, isOutput=False)
    one_e = nc.declare_dram_parameter("ones", [128, 128], bdt, isOutput=False)

    y_e = nc.declare_dram_parameter("y", [SHARD, Dc], fp32, isOutput=True)

    # A2A staging per batch: block j = [512 local feats, 256 toks of batch-b
    # token-chunk j]. Splitting per batch lets A2A(b0) overlap b1 compute.
    TPB = SHARD // Bc                    # tokens per core per batch (256)
    yatt = [nc.dram_tensor(f"yatt{b}", [n_cores * HDLc, TPB], bdt)
            for b in range(Bc)]
    zatt = [nc.dram_tensor(f"zatt{b}", [n_cores * HDLc, TPB], bdt)
            for b in range(Bc)]

    def mm(ps, lhsT, rhs, start, stop):
        nc.tensor.matmul(ps, lhsT, rhs, start=start, stop=stop)

    with tile.TileContext(nc) as tc:
        with (
            tc.tile_pool(name="const", bufs=1) as constp,
            tc.tile_pool(name="zsb", bufs=1) as zp,
        ):
            zt = {}

            def load_z(b):
                # z(b) = token-sharded attention output, feature-major; DMA'd
                # as soon as A2A(b) lands (the pool lives outside the compute
                # pools so these loads never wait on attention SBUF reuse)
                zt[b] = []
                for q in range(NQ):
                    t = zp.tile([128, DCQ, TPB], bdt, tag=f"z{b}{q}",
                                name=f"z{b}{q}")
                    # gpsimd queue: these gather-pattern DMAs cost ~2-6us of
                    # descriptor writes each; on sync they head-block the
                    # weight-stream triggers at the phase transitions
                    nc.gpsimd.dma_start(
                        t[:],
                        zatt[b][q * 1024:(q + 1) * 1024, :]
                        .rearrange("(j p) f -> p j f", p=128))
                    zt[b].append(t)

            with (
                tc.tile_pool(name="qkv", bufs=1) as qkvp,
                tc.tile_pool(name="hst", bufs=6) as hstp,
                tc.tile_pool(name="wst", bufs=4) as wstp,
                tc.tile_pool(name="rope", bufs=2) as ropep,
                tc.tile_pool(name="ptsb", bufs=8) as ptp,
                tc.tile_pool(name="rcp", bufs=2) as rcpp,
                tc.tile_pool(name="atnsb", bufs=2) as atnp,
            ):
                def load_hq(b, sc):
                    hq = []
                    for q in range(NQ):
                        ht = hstp.tile([128, DCQ, SC], bdt, tag="h", name=f"h{q}")
                        nc.sync.dma_start(ht[:], hsT_e[b, sc, q])
                        hq.append(ht)
                    return hq

                # rolling weight-quarter prefetcher: the whole phase-A program
                # consumes (wq, wk, wv) x 4 quarters per (b, sc) in a fixed
                # order; keep 2 quarters in flight ahead of the consumer so a
                # pass never waits on its weight DMA (and b1's first quarter
                # is issued during b0's attention, ahead of the attention
                # output DMAs in the queues).
                from collections import deque
                w_seq = deque()
                for b in range(Bc):
                    for sc in range(NSC):
                        for w_e in (wqT_e, wkT_e, wvT_e):
                            for q in range(NQ):
                                w_seq.append((w_e, q))
                w_pending = deque()

                def w_issue():
                    if not w_seq:
                        return
                    w_e, q = w_seq.popleft()
                    wt = wstp.tile([128, DCQ, HDLc], bdt, tag="w")
                    nc.sync.dma_start(wt[:], w_e[q])
                    w_pending.append(wt)

                def w_pop():
                    w_issue()
                    return w_pending.popleft()

                # interleave the first hst quarters with the first weight
                # quarters so each projection quarter's dependency pair (h_q +
                # w_q) lands in consumption order in the in-order DMA queues;
                # the very first quarter is split in half so the opening
                # matmuls start after 1.5 MiB instead of 2 MiB
                h0ab = []
                for hh in range(2):
                    t = hstp.tile([128, DCQ // 2, SC], bdt, tag="h",
                                  name=f"h0{hh}")
                    nc.sync.dma_start(
                        t[:], hsT_e[0, 0, 0][:, hh * 4:(hh + 1) * 4, :])
                    h0ab.append(t)
                    if hh == 0:
                        w_issue()
                first_hq = [("split", h0ab)]
                for q in range(1, NQ):
                    ht = hstp.tile([128, DCQ, SC], bdt, tag="h", name=f"h{q}")
                    nc.sync.dma_start(ht[:], hsT_e[0, 0, q])
                    first_hq.append(ht)
                    if q < 3:
                        w_issue()
                next_hq = first_hq
                # const tiles allocated here, but their DMAs are emitted after
                # the first Q-pass so the opening weight stream isn't queued
                # behind them (cos/sin are first needed ~45us in, masks/ones
                # only at attention). pswap is dead since RoPE moved to DMA.
                masksT = constp.tile([128, 128], fp32)
                ones_t = constp.tile([128, 128], bdt)
                cosb = [constp.tile([ROTc, Sc], bdt, name=f"cos{b}")
                        for b in range(Bc)]
                sinb = [constp.tile([ROTc, Sc], bdt, name=f"sin{b}")
                        for b in range(Bc)]

                def load_consts():
                    nc.sync.dma_start(masksT[:], msk_e[:])
                    nc.sync.dma_start(ones_t[:], one_e[:])
                    for bb in range(Bc):
                        nc.sync.dma_start(cosb[bb][:], cos_e[bb])
                        nc.sync.dma_start(sinb[bb][:], sin_e[bb])

                # out-proj weight blocks are loaded into freed hst/wst pool
                # buffers (identical tile shape) — no extra SBUF. Splitting
                # across both pools gives 10 effective buffers, enough to
                # fully double-buffer a 4-tile block per phase-C visit.
                wo_tiles = {}

                def load_wo(ob):
                    tiles = []
                    for q in range(NQ):
                        pool, tg = (hstp, "h") if q < 2 else (wstp, "w")
                        w4 = pool.tile([128, DCQ, SC], bdt, tag=tg,
                                       name=f"wo{q}")
                        nc.sync.dma_start(
                            w4[:], woT_e[ob, :, q * DCQ:(q + 1) * DCQ, :])
                        tiles.append(w4)
                    return tiles

                for b in range(Bc):
                    QT = [qkvp.tile([128, Sc], bdt, tag=f"QT{c}", name=f"QT{c}")
                          for c in range(NHC)]
                    KT = [qkvp.tile([128, Sc], bdt, tag=f"KT{c}", name=f"KT{c}")
                          for c in range(NHC)]
                    V = [qkvp.tile([128, HDLc], bdt, tag=f"V{k}", name=f"V{k}")
                         for k in range(NKC)]

                    # ---------------- phase A: QKV projection + RoPE ----------------
                    with tc.tile_pool(name="pjps", bufs=2, space="PSUM") as pjps:
                        for sc in range(NSC):
                            ssl = slice(sc * SC, (sc + 1) * SC)
                            hq = next_hq
                            if sc + 1 < NSC:
                                next_hq = load_hq(b, sc + 1)
                            elif b + 1 < Bc:
                                next_hq = load_hq(b + 1, 0)

                            def hqs(q, dc, csl=slice(0, SC)):
                                ent = hq[q]
                                if isinstance(ent, tuple):
                                    return ent[1][dc // 4][:, dc % 4, csl]
                                return ent[:, dc, csl]

                            # RoPE on rot rows: the adjacent-partition swap is
                            # done by two SBUF->SBUF DMAs on the idle scalar
                            # queue (the data is already bf16, so this is
                            # exact and frees ~10us of PE matmuls), then DVE
                            # mul/add. Emitted one quarter INTO the following
                            # pass so the serial DVE chain hides under that
                            # pass's matmuls instead of stalling attention.
                            def rope_block(T, ti):
                                for hch in range(0, NHC, HDc // 128):
                                    sw = ropep.tile([ROTc, SC], bdt, tag="sw")
                                    src = (T[hch][0:ROTc, ssl]
                                           .rearrange("(a two) f -> a two f",
                                                      two=2))
                                    dst = sw[:].rearrange(
                                        "(a two) f -> a two f", two=2)
                                    nc.scalar.dma_start(dst[:, 0, :],
                                                        src[:, 1, :])
                                    nc.scalar.dma_start(dst[:, 1, :],
                                                        src[:, 0, :])
                                    t1 = ropep.tile([ROTc, SC], bdt, tag="t1")
                                    t2 = ropep.tile([ROTc, SC], bdt, tag="t2")
                                    nc.vector.tensor_tensor(
                                        t1[:], sw[:], sinb[b][:, ssl],
                                        op=mybir.AluOpType.mult)
                                    nc.vector.tensor_tensor(
                                        t2[:], T[hch][0:ROTc, ssl], cosb[b][:, ssl],
                                        op=mybir.AluOpType.mult)
                                    nc.vector.tensor_add(T[hch][0:ROTc, ssl],
                                                         t1[:], t2[:])

                            # Q and K passes: out [hd=128, s=512] per head-chunk
                            for pi, (w_e, T) in enumerate(((wqT_e, QT),
                                                           (wkT_e, KT))):
                                ps = [pjps.tile([128, SC], fp32, tag=f"pj{i}",
                                                name=f"pj{i}")
                                      for i in range(NHC)]
                                for q in range(NQ):
                                    wt = w_pop()
                                    for dc in range(DCQ):
                                        for hc in range(NHC):
                                            mm(ps[hc][:],
                                               wt[:, dc, hc * 128:(hc + 1) * 128],
                                               hqs(q, dc),
                                               start=(q == 0 and dc == 0),
                                               stop=(q == NQ - 1 and dc == DCQ - 1))
                                    if q == 0 and pi == 1:
                                        rope_block(QT, 0)   # hides in K pass
                                    if q == NQ - 1 and pi == 0 \
                                            and b == 0 and sc == 0:
                                        load_consts()
                                # alternate engines: the next pass's PSUM
                                # buffer rotation waits on these evacuations
                                for hc in range(NHC):
                                    if hc % 2 == 0:
                                        nc.scalar.copy(T[hc][:, ssl],
                                                       ps[hc][:])
                                    else:
                                        nc.vector.tensor_copy(T[hc][:, ssl],
                                                              ps[hc][:])

                            # V pass: out [s=128, hdl=512] per token subtile.
                            # Tags offset by 2: the ropeQ sw tiles took pj0/pj1
                            # (Q-pass buffers); mapping ts 0/1 onto pj2/pj3
                            # (also Q-pass buffers, long free) keeps the first
                            # V matmuls off the just-retired K-pass buffers,
                            # whose scalar-copy evacuations are still running.
                            ps = [pjps.tile([128, HDLc], fp32,
                                            tag=f"pj{(i + 2) % 4}",
                                            name=f"pv{i}")
                                  for i in range(NHC)]
                            for q in range(NQ):
                                wt = w_pop()
                                for dc in range(DCQ):
                                    for ts in range(4):
                                        mm(ps[ts][:],
                                           hqs(q, dc,
                                               slice(ts * 128, (ts + 1) * 128)),
                                           wt[:, dc, :],
                                           start=(q == 0 and dc == 0),
                                           stop=(q == NQ - 1 and dc == DCQ - 1))
                                if q == 0:
                                    rope_block(KT, 1)       # hides in V pass
                            # alternate engines: these four evacuations gate
                            # the pjps pool-close barrier ahead of attention
                            for ts in range(4):
                                if ts % 2 == 0:
                                    nc.vector.tensor_copy(V[sc * 4 + ts][:],
                                                          ps[ts][:])
                                else:
                                    nc.scalar.copy(V[sc * 4 + ts][:], ps[ts][:])

                    # ---------------- phase B: attention ----------------
                    if b == 1:
                        load_z(0)   # A2A(b0) is done by now; overlaps b1 work
                        # prefetch the first two out-proj weight blocks during
                        # b1's attention so phase C starts without DMA stalls;
                        # the tiles reuse freed hst/wst pool buffers
                        wo_tiles[0] = load_wo(0)
                        wo_tiles[1] = load_wo(1)
                    with (
                        tc.tile_pool(name="ssps", bufs=3, space="PSUM") as ssps,
                        tc.tile_pool(name="atps0", bufs=2, space="PSUM") as atps0,
                        tc.tile_pool(name="atps1", bufs=1, space="PSUM") as atps1,
                        tc.tile_pool(name="rsps", bufs=2, space="PSUM") as rsps,
                    ):
                        for h in range(HPCc):
                            c0, c1 = 2 * h, 2 * h + 1
                            for qm in range(NSC):
                                nkc = 4 * (qm + 1)
                                npair = nkc // 2
                                LAG = 3          # PV lag in kc-pairs

                                # diagonal trimming: for the last 4 k-chunks,
                                # q-columns below the diagonal block are fully
                                # masked — skip them. Computed region of kc =
                                # absolute q [off(kc), 512): off = 128 *
                                # (kc - (nkc-4)) for diagonal kcs else 0.
                                def off(kc, _n=nkc):
                                    return max(0, (kc - (_n - 4))) * 128

                                at_ps = [
                                    atps0.tile([128, SC], fp32, tag="at0",
                                               name="at0"),
                                    atps1.tile([128, SC], fp32, tag="at1",
                                               name="at1"),
                                ]
                                rs_ps = rsps.tile([128, SC], fp32, tag="rs")
                                pts = {}
                                # kc processed in pairs, banks interleaved so
                                # no two consecutive matmuls hit the same
                                # PSUM bank; PV lags LAG pairs behind so exp
                                # (ACT) latency stays off the PE critical
                                # path. Row sums via ones-stationary matmul
                                # (result broadcast across partitions free).
                                for step in range(npair + LAG):
                                    if step < npair:
                                        k0, k1 = 2 * step, 2 * step + 1
                                        o0, o1 = off(k0), off(k1)
                                        w0, w1 = SC - o0, SC - o1
                                        l0 = slice(k0 * 128, (k0 + 1) * 128)
                                        l1 = slice(k1 * 128, (k1 + 1) * 128)
                                        q0 = slice(qm * SC + o0, (qm + 1) * SC)
                                        q1 = slice(qm * SC + o1, (qm + 1) * SC)
                                        s0 = ssps.tile([128, SC], fp32, tag="ss",
                                                       name="ss0")
                                        s1 = ssps.tile([128, SC], fp32, tag="ss",
                                                       name="ss1")
                                        mm(s0[:, 0:w0], KT[c0][:, l0],
                                           QT[c0][:, q0], start=True, stop=False)
                                        mm(s1[:, 0:w1], KT[c0][:, l1],
                                           QT[c0][:, q1], start=True, stop=False)
                                        mm(s0[:, 0:w0], KT[c1][:, l0],
                                           QT[c1][:, q0], start=False, stop=True)
                                        mm(s1[:, 0:w1], KT[c1][:, l1],
                                           QT[c1][:, q1], start=False, stop=True)
                                        for kc, ss in ((k0, s0), (k1, s1)):
                                            o, w = off(kc), SC - off(kc)
                                            if kc >= nkc - 4:
                                                # triangle sits in the first
                                                # 128 computed columns
                                                nc.vector.tensor_add(
                                                    ss[:, 0:128], ss[:, 0:128],
                                                    masksT[:])
                                            pt = ptp.tile([128, SC], bdt,
                                                          tag="pt")
                                            nc.scalar.activation(
                                                pt[:, 0:w], ss[:, 0:w],
                                                mybir.ActivationFunctionType.Exp,
                                                bias=0.0, scale=1.0 / 16.0)
                                            pts[kc] = pt
                                    if step >= LAG:
                                        for kc in (2 * (step - LAG),
                                                   2 * (step - LAG) + 1):
                                            pt = pts.pop(kc)
                                            o, w = off(kc), SC - off(kc)
                                            st = (kc == 0)
                                            sp = (kc == nkc - 1)
                                            mm(rs_ps[:, o:], ones_t[:],
                                               pt[:, 0:w], start=st, stop=sp)
                                            mm(at_ps[0][:, o:],
                                               V[kc][:, h * HDc:h * HDc + 128],
                                               pt[:, 0:w], start=st, stop=sp)
                                            mm(at_ps[1][:, o:],
                                               V[kc][:, h * HDc + 128:(h + 1) * HDc],
                                               pt[:, 0:w], start=st, stop=sp)
                                recip = rcpp.tile([128, SC], fp32, tag="rc")
                                # exact reciprocal costs 3.4us on DVE and
                                # head-blocks the mask-adds/normalizes queued
                                # behind it at every qm boundary; the ~18-bit
                                # approx is 5x faster and the denominator is a
                                # benign [1, ~5e2] softmax rowsum
                                nc.vector.reciprocal_approx_fast(
                                    recip[:], rs_ps[:])
                                for hh in range(2):
                                    atn = atnp.tile([128, SC], bdt, tag=f"atn{hh}")
                                    nc.vector.tensor_tensor(
                                        atn[:], at_ps[hh][:], recip[:],
                                        op=mybir.AluOpType.mult)
                                    row0 = h * HDc + hh * 128
                                    # split across the two destination cores
                                    # covering this 512-token q block; gpsimd
                                    # queue (same as the consuming A2A) keeps
                                    # these off the sync queue's weight-stream
                                    # triggers
                                    for half in range(2):
                                        j = 2 * qm + half
                                        nc.gpsimd.dma_start(
                                            yatt[b][j * HDLc + row0:
                                                    j * HDLc + row0 + 128, :],
                                            atn[:, half * TPB:(half + 1) * TPB])

                    # A2A(b): head-sharded -> token-sharded; b0's overlaps
                    # b1's QKV/attention compute entirely.
                    nc.gpsimd.collective_compute(
                        "AllToAll",
                        mybir.AluOpType.bypass,
                        replica_groups=[list(range(n_cores))],
                        ins=[yatt[b][:]],
                        outs=[zatt[b][:]],
                    )
                    if b == 1:
                        # z(b1) pull is dependency-tracked on the A2A write;
                        # issuing it here puts it ahead of phase C's weight
                        # streaming in the DMA queues
                        load_z(1)

                # ---------------- phase C: out projection ----------------
                # Wo column-blocks mostly stream ONCE and serve both batches,
                # but the A2A(b1) collective takes ~40us after b1's attention
                # drains — so the first DEFER obs visit only b0's tokens
                # (their z landed with A2A(b0) long ago) and their b1 halves
                # run at the very end (re-streaming just DEFER x 4 MiB of Wo).
                DEFER = 4
                visits = ([(ob, (0,)) for ob in range(DEFER)]
                          + [(ob, (0, 1)) for ob in range(DEFER, NOB)]
                          + [(ob, (1,)) for ob in range(DEFER)])
                with (
                    tc.tile_pool(name="ysb", bufs=4) as ysbp,
                    tc.tile_pool(name="yps", bufs=4, space="PSUM") as ypsp,
                ):
                    for vi, (ob, bs) in enumerate(visits):
                        ocl = slice(ob * SC, (ob + 1) * SC)
                        woq = (wo_tiles.pop(ob) if ob in wo_tiles
                               else load_wo(ob))
                        if vi + 1 < len(visits):
                            nob = visits[vi + 1][0]
                            if nob not in wo_tiles:
                                wo_tiles[nob] = load_wo(nob)
                        for b in bs:
                            z = zt[b]
                            # 2 token-tile accumulation chains interleaved so
                            # consecutive matmuls hit different PSUM banks
                            yp = [ypsp.tile([128, SC], fp32, tag="yp",
                                            name=f"yp{tt}")
                                  for tt in range(TPB // 128)]
                            for fc in range(NFC):
                                for tt in range(TPB // 128):
                                    mm(yp[tt][:],
                                       z[fc // DCQ][:, fc % DCQ,
                                                    tt * 128:(tt + 1) * 128],
                                       woq[fc // DCQ][:, fc % DCQ, :],
                                       start=(fc == 0), stop=(fc == NFC - 1))
                            for tt in range(TPB // 128):
                                tsl = slice(b * TPB + tt * 128,
                                            b * TPB + (tt + 1) * 128)
                                ysb = ysbp.tile([128, SC], fp32, tag="ysb")
                                if tt % 2 == 0:
                                    nc.scalar.copy(ysb[:], yp[tt][:])
                                else:
                                    nc.vector.tensor_copy(ysb[:], yp[tt][:])
                                nc.sync.dma_start(y_e[tsl, ocl], ysb[:])

    nc.compile()
    return nc


# ---------------------------------------------------------------- host prep

def _sinusoidal_np(num_pos, dim):
    inv_freq = 1.0 / (10000.0 ** (np.arange(0, dim, 2, dtype=np.float32) / dim))
    t = np.arange(num_pos, dtype=np.float32)[:, None] * inv_freq[None, :]
    return np.cos(t).astype(np.float32), np.sin(t).astype(np.float32)  # [P, dim//2]


def _host_arrays(hs, Wq, Wk, Wv, Wo, position_ids, cfg, n_cores):
    """Build the shared + per-core input arrays."""
    import ml_dtypes
    bf16 = ml_dtypes.bfloat16

    Bc, Sc, Dc, HPCc, HDc, ROTc = (
        cfg["B"], cfg["S"], cfg["D"], cfg["HPC"], cfg["HD"], cfg["ROT"])
    HDLc = HPCc * HDc
    NSC, NQ = Sc // SC, 4
    DCQ = Dc // NQ // 128
    NOB, NFC = Dc // SC, Dc // 128
    # hsT pre-tiled to [B, sc, q, 128, dcq, 512] (SBUF tile layout)
    hsT = np.ascontiguousarray(hs.transpose(0, 2, 1)).astype(bf16)  # [B, D, S]
    hsT = hsT.reshape(Bc, NQ, DCQ, 128, NSC, SC).transpose(0, 4, 1, 3, 2, 5)
    hsT = np.ascontiguousarray(hsT)

    def tile_w(wT):      # [D, 512] -> [q, 128, dcq, 512]
        return np.ascontiguousarray(
            wT.reshape(NQ, DCQ, 128, HDLc).transpose(0, 2, 1, 3))

    cos_t, sin_t = _sinusoidal_np(max(MAX_POS, Sc), ROTc)   # [P, ROT//2]
    pos = np.asarray(position_ids).astype(np.int64)         # [B, S]
    cosg = cos_t[pos]                                       # [B, S, 32]
    sing = sin_t[pos]
    cosb = np.repeat(cosg.transpose(0, 2, 1), 2, axis=1)    # [B, 64, S]
    sinb_r = np.repeat(sing.transpose(0, 2, 1), 2, axis=1)
    sgn = np.ones((ROTc, 1), np.float32)
    sgn[0::2] = -1.0
    sinb = np.ascontiguousarray(sinb_r * sgn).astype(bf16)
    cosb = np.ascontiguousarray(cosb).astype(bf16)

    # transposed causal triangle for a diagonal 128x128 block:
    # masksT[r, c] = 0 if r <= c else NEG  (k_local <= q_local)
    kk = np.arange(128)[:, None]
    qq = np.arange(128)[None, :]
    masksT = np.where(kk <= qq, 0.0, NEG).astype(np.float32)

    pswap = np.zeros((128, ROTc), np.float32)
    for f in range(ROTc // 2):
        pswap[2 * f + 1, 2 * f] = 1.0
        pswap[2 * f, 2 * f + 1] = 1.0
    pswap = pswap.astype(bf16)
    ones = np.ones((128, 128), np.float32).astype(bf16)

    woT = np.ascontiguousarray(np.asarray(Wo, np.float32).T).astype(bf16)
    # pre-tiled to [ob, 128, fc, 512]
    woT = np.ascontiguousarray(
        woT.reshape(NFC, 128, NOB, SC).transpose(2, 1, 0, 3))

    shared = dict(hsT=hsT, cosb=cosb, sinb=sinb, masksT=masksT,
                  pswap=pswap, ones=ones, woT=woT)
    per_core = []
    for c in range(n_cores):
        csl = slice(c * HDLc, (c + 1) * HDLc)
        per_core.append(dict(
            wqT=tile_w(np.ascontiguousarray(Wq[csl, :].T).astype(bf16)),
            wkT=tile_w(np.ascontiguousarray(Wk[csl, :].T).astype(bf16)),
            wvT=tile_w(np.ascontiguousarray(Wv[csl, :].T).astype(bf16)),
            **shared,
        ))
    return per_core


def _numpy_reference(hidden_states, Wq, Wk, Wv, Wo, layer_past_k, layer_past_v,
                     attention_mask, position_ids, new_key_loc, new_value_loc,
                     valid_key_indices, valid_value_indices, bucket_size):
    """Slow but general fallback (mirrors reference.py in numpy fp32)."""
    hs = np.asarray(hidden_states, np.float32)
    Bc, Sc, Dc = hs.shape
    q = (hs @ np.asarray(Wq).T).reshape(Bc, Sc, NH, HD)
    k = (hs @ np.asarray(Wk).T).reshape(Bc, Sc, NH, HD)
    v = (hs @ np.asarray(Wv).T).reshape(Bc, Sc, NH, HD)

    cos_t, sin_t = _sinusoidal_np(MAX_POS, ROT)
    pos = np.asarray(position_ids).astype(np.int64)
    c_ = cos_t[pos][:, :, None, :]      # [B,S,1,32]
    s_ = sin_t[pos][:, :, None, :]

    def rot(x):
        xr = x[..., :ROT].reshape(Bc, Sc, NH, ROT // 2, 2)
        x0, x1 = xr[..., 0], xr[..., 1]
        o0 = c_ * x0 - s_ * x1
        o1 = s_ * x0 + c_ * x1
        out = np.stack([o0, o1], axis=-1).reshape(Bc, Sc, NH, ROT)
        return np.concatenate([out, x[..., ROT:]], axis=-1)

    q, k = rot(q), rot(k)
    nk = np.asarray(layer_past_k, np.float32).copy()
    nv = np.asarray(layer_past_v, np.float32).copy()
    nk[np.asarray(new_key_loc)] = k.reshape(Bc * Sc, 1, NH, HD)
    nv[np.asarray(new_value_loc)] = v.reshape(Bc * Sc, 1, NH, HD)
    kg = nk[np.asarray(valid_key_indices)].reshape(
        Bc, bucket_size, NH, HD).transpose(0, 2, 1, 3)
    vg = nv[np.asarray(valid_value_indices)].reshape(
        Bc, bucket_size, NH, HD).transpose(0, 2, 1, 3)
    qh = q.transpose(0, 2, 1, 3)
    scores = np.einsum("bhqd,bhkd->bhqk", qh, kg)
    causal = np.tril(np.ones((MAX_POS, MAX_POS), bool))[
        bucket_size - Sc:bucket_size, :bucket_size]
    scores = np.where(causal, scores, np.float32(np.finfo(np.float32).min))
    scores = scores / np.float32(np.sqrt(HD)) + np.asarray(attention_mask, np.float32)
    scores = scores - scores.max(-1, keepdims=True)
    p = np.exp(scores)
    p = p / p.sum(-1, keepdims=True)
    attn = np.einsum("bhqk,bhkd->bhqd", p, vg)
    attn = attn.transpose(0, 2, 1, 3).reshape(Bc, Sc, Dc)
    return (attn @ np.asarray(Wo).T).astype(np.float32)


def _fast_path_ok(layer_past_k, layer_past_v, attention_mask, new_key_loc,
                  new_value_loc, valid_key_indices, valid_value_indices,
                  bucket_size, hs_shape):
    Bc, Sc, Dc = hs_shape
    if (Bc, Sc, Dc) != (B, S, D) or int(bucket_size) != S:
        return False
    ar = np.arange(Bc * Sc)
    for idx in (new_key_loc, new_value_loc, valid_key_indices, valid_value_indices):
        a = np.asarray(idx)
        if a.shape != (Bc * Sc,) or not np.array_equal(a, ar):
            return False
    if np.any(np.asarray(attention_mask) != 0):
        return False
    return True


_NC_CACHE = {}


def _get_nc(use_collective=True):
    key = "full"
    if key not in _NC_CACHE:
        _NC_CACHE[key] = build_nc(_cfg_full(), n_cores=N_CORES)
    return _NC_CACHE[key]


def kernel(**inputs):
    hs = np.asarray(inputs["hidden_states"], np.float32)
    fast = _fast_path_ok(
        inputs["layer_past_k"], inputs["layer_past_v"], inputs["attention_mask"],
        inputs["new_key_loc"], inputs["new_value_loc"],
        inputs["valid_key_indices"], inputs["valid_value_indices"],
        inputs["bucket_size"], hs.shape)
    if not fast:
        return _numpy_reference(**inputs)

    from concourse.bass_utils import run_bass_kernel_spmd

    nc = _get_nc(True)
    in_maps = _host_arrays(
        hs, np.asarray(inputs["Wq"], np.float32),
        np.asarray(inputs["Wk"], np.float32),
        np.asarray(inputs["Wv"], np.float32),
        np.asarray(inputs["Wo"], np.float32),
        inputs["position_ids"], _cfg_full(), N_CORES)
    res = run_bass_kernel_spmd(nc, in_maps, list(range(N_CORES)))
    outs = [res.results[c]["y"] for c in range(N_CORES)]
    return _unshard(outs)


def _unshard(outs):
    """Core c's [512, D] shard = [b0 tokens 256c:256(c+1); b1 same range]."""
    tpb = (B * S) // N_CORES // B        # 256
    y = np.empty((B, S, D), np.float32)
    for c, o in enumerate(outs):
        for b in range(B):
            y[b, c * tpb:(c + 1) * tpb] = o[b * tpb:(b + 1) * tpb]
    return y

